# revision 37
# baseline (speedup 1.0000x reference)
"""Bass/Tile TRN2 kernel for per-model-batched causal self-attention.

Problem: x[M,B,S,D], qkv_w[M,D,3D], proj_w[M,D,D] -> out[M,B,S,D]
M=8 models sharded across 8 NeuronCores (embarrassingly parallel).

Per-core design (model m), per batch b:
  xT      = PE-transpose(x_b)  (f32r)               [D,S]
  qkT     = wqkv[:, :1024].T-proj (fp32r matmul)    [1024,S] -> bf16 (q^T,k^T)
  V       = x @ wqkv[:, 1024:] (fp32r)              [S,512] -> bf16, +ones col
  st[k,q] = K @ Q^T  (bf16, causal-trimmed)         PSUM f32
  p       = exp(st/8) (ScalarE), diag blocks masked by tri01 mul (Pool)
  y_aug   = p.T @ V_aug (bf16)  -> y[q,d] + softmax sums in col 64 (PSUM)
  y       = y_aug * (1/sums)  row-broadcast multiply (DVE)
  ynT     = PE-transpose(y) (f32r)                  [D,S]
  out     = ynT.T @ wproj (fp32r)

Schedule: score groups run two head-groups ahead of the y phase (se
pipeline) so the ScalarE exp chain never gates the PE; the next batch's
load/transpose/projection groups are interleaved as filler between
score/y units via a keyed work queue (ensure() forces prerequisites
before a score group that consumes them, keeping the in-order PE stream
deadlock-free).  Batch 0 runs a DMA-pipelined prologue: weight chunks
arrive in qkT-group order and all four score groups are emitted before
the first y so the PE stays fed while the V weight columns are still in
flight.  Engine split (GPSIMD cannot touch PSUM on TRN2): DVE does
psum->sbuf copies, reciprocal and the normalize; Act does exp, yT and
out copies; Pool does the SBUF-only causal mask multiplies.  PSUM:
score/y tiles share one 3-deep ring (6 banks) + 2 banks for the
projection/transpose ring.
"""

import sys

if "/opt/trn_rl_repo" not in sys.path:
    sys.path.insert(0, "/opt/trn_rl_repo")

from contextlib import nullcontext
from functools import partial

import numpy as np

import concourse.bass as bass
import concourse.mybir as mybir
import concourse.tile as tile
from concourse import bacc, bass_utils
from concourse.masks import (
    make_identity,
    make_lower_triangular,
    make_upper_triangular,
)

M, B, S, D, H = 8, 4, 512, 512, 8
HD = D // H  # 64
F32 = mybir.dt.float32
F32R = mybir.dt.float32r
BF16 = mybir.dt.bfloat16

N_CORES = 8

# engine assignment for the movable ops (sweepable)
# NOTE: GPSIMD (Pool) cannot access PSUM on TRN2 hardware -- psum-touching
# ops may only go on "vector" (DVE) or "scalar" (Act).
CFG = {
    "norm": "vector",   # y normalization tensor_scalar_mul (reads PSUM)
    "v": "vector",      # V psum->sbuf copy
    "yt": "scalar",     # yT psum->sbuf copy
    "ob": "vector",     # out psum->sbuf copy
    "preload": "vector",  # causal-bias psum preload
    "mask": "post",  # "preload" (pre-matmul psum bias) or "post" (tri mul)
    "mask_eng": "gpsimd",
    "norm_bcast": True,
    "paced": False,
    "ysb_bf16": False,
    "ahead": 2,
    "yt_defer": True,
    "y_first": True,
    "xt": "vector",    # xT psum->sbuf copy
    "qk": "vector",    # qkT psum->sbuf copy (alt: "alt" = alternate DVE/Act)
    "psum": "shared",  # "shared" (stp+yp one 3-deep ring) or "split"
}

_cache = {}


def _copy(nc, eng, out, in_):
    if eng == "scalar":
        nc.scalar.copy(out=out, in_=in_)
    else:
        getattr(nc, eng).tensor_copy(out=out, in_=in_)


def build_nc(reps=1):
    nc = bacc.Bacc("TRN2", target_bir_lowering=False, debug=False)

    x_d = nc.dram_tensor("x", [B, S, D], F32, kind="ExternalInput")
    wqkv_d = nc.dram_tensor("wqkv", [D, 3 * D], F32, kind="ExternalInput")
    wproj_d = nc.dram_tensor("wproj", [D, D], F32, kind="ExternalInput")
    out_d = nc.dram_tensor("out", [B, S, D], F32, kind="ExternalOutput")

    with tile.TileContext(nc) as tc:
        with (
            tc.tile_pool(name="singles", bufs=1) as singles,
            tc.tile_pool(name="xp", bufs=2) as xpool,
            tc.tile_pool(name="xtp", bufs=2) as xtpool,
            tc.tile_pool(name="qk", bufs=2) as qkpool,
            tc.tile_pool(name="vp", bufs=2) as vpool,
            tc.tile_pool(name="se", bufs=4) as sepool,
            tc.tile_pool(name="yp", bufs=2) as ypool,
            tc.tile_pool(name="ytp", bufs=3) as ytpool,
            tc.tile_pool(name="op", bufs=2) as opool,
            tc.tile_pool(name="rp", bufs=4) as rpool,
            tc.tile_pool(name="ps_mm", bufs=2, space=bass.MemorySpace.PSUM) as ps_mm,
            tc.tile_pool(
                name="ps_att",
                bufs=3 if CFG["psum"] == "shared" else 2,
                space=bass.MemorySpace.PSUM,
            ) as ps_att,
            tc.tile_pool(name="ps_y", bufs=1, space=bass.MemorySpace.PSUM) as ps_y,
        ):
          with tc.For_i(0, reps, 1) if reps > 1 else nullcontext():
            # ---- constants ----
            ident = singles.tile([128, 128], F32)
            make_identity(nc, ident[:])
            ident_r = singles.tile([128, 128], F32R)
            nc.vector.tensor_copy(out=ident_r[:], in_=ident[:])
            ident_b = singles.tile([128, 128], BF16)
            nc.vector.tensor_copy(out=ident_b[:], in_=ident[:])
            # strict-lower (k>q) = -30000 bias tile, x2 heads; accumulated
            # into the diag-block PSUM before the score matmul
            trib = singles.tile([128, 2, 128], F32)
            make_lower_triangular(nc, trib[:, 0, :], val=-30000.0, diag=False)
            nc.gpsimd.tensor_copy(out=trib[:, 1, :], in_=trib[:, 0, :])
            tri2 = singles.tile([128, 2, 128], BF16)  # keep-mask (k<=q), x2
            make_upper_triangular(nc, tri2[:, 0, :], val=1.0, diag=True)
            nc.gpsimd.tensor_copy(out=tri2[:, 1, :], in_=tri2[:, 0, :])

            wqk = [singles.tile([128, 4, 128], F32R, name=f"wqk{mt}") for mt in range(8)]
            wv = singles.tile([128, 4, D], F32R)
            wproj = singles.tile([128, 4, D], F32R)

            state = {}

            # ---------- stage A (loads + projections), as schedulable groups ----
            def emit_load_x(b):
                x_sb = [
                    xpool.tile([128, D], F32R, tag=f"x{stq}", name=f"xsb{stq}")
                    for stq in range(4)
                ]
                for stq in range(4):
                    nc.sync.dma_start(
                        out=x_sb[stq][:],
                        in_=x_d.ap().bitcast(F32R)[b][
                            stq * 128 : (stq + 1) * 128, :
                        ],
                    )
                v_sb = vpool.tile([128, 4, H, 66], BF16, tag="v", name="vsb")
                nc.gpsimd.memset(v_sb[:, :, :, 64:65], 1.0)
                xt = xtpool.tile([128, 4, S], F32R, tag="xt", name="xt")
                state[b] = {"x": x_sb, "xT": xt, "qkT": {}, "v": v_sb, "ynT": []}
                if b == 0:
                    # qk weight columns in per-mt tiles ordered to pipeline
                    # with the qkT projection groups; v columns + wproj last
                    for mt in (0, 4, 1, 5, 2, 6, 3, 7):
                        nc.sync.dma_start(
                            out=wqk[mt][:],
                            in_=wqkv_d.ap()
                            .bitcast(F32R)[:, mt * 128 : (mt + 1) * 128]
                            .rearrange("(c p) o -> p c o", p=128),
                        )
                    nc.sync.dma_start(
                        out=wv[:],
                        in_=wqkv_d.ap().bitcast(F32R)[:, 1024:1536].rearrange(
                            "(c p) o -> p c o", p=128
                        ),
                    )
                    nc.sync.dma_start(
                        out=wproj[:],
                        in_=wproj_d.ap()
                        .bitcast(F32R)
                        .rearrange("(c p) o -> p c o", p=128),
                    )

            def emit_xt_group(b, st):
                # transpose the 4 d-blocks of s-rows [st*128, (st+1)*128)
                st_ = state[b]
                tp = ps_mm.tile([128, 4, 128], F32, tag="mm", name="tpx")
                for dc in range(4):
                    nc.tensor.transpose(
                        tp[:, dc, :].bitcast(F32R),
                        st_["x"][st][:, dc * 128 : (dc + 1) * 128],
                        ident_r[:],
                    )
                _copy(
                    nc, CFG["xt"],
                    st_["xT"][:, :, st * 128 : (st + 1) * 128], tp[:]
                )

            def emit_qkt_group(b, mt):
                st_ = state[b]
                xt = st_["xT"]
                mp = ps_mm.tile([128, 512], F32, tag="mm", name="mp")
                for dc in range(4):
                    nc.tensor.matmul(
                        mp[:],
                        wqk[mt][:, dc, :],
                        xt[:, dc, :],
                        start=(dc == 0),
                        stop=(dc == 3),
                    )
                qk = qkpool.tile([128, 512], BF16, tag=f"qk{mt}", name=f"qk{mt}")
                qeng = CFG["qk"] if CFG["qk"] != "alt" else ("vector", "scalar")[mt % 2]
                _copy(nc, qeng, qk[:], mp[:])
                st_["qkT"][mt] = qk

            def emit_v_group(b, stt):
                st_ = state[b]
                xt = st_["xT"]
                vp_ps = ps_mm.tile([128, 512], F32, tag="mm", name="vp")
                for dc in range(4):
                    nc.tensor.matmul(
                        vp_ps[:],
                        xt[:, dc, stt * 128 : (stt + 1) * 128],
                        wv[:, dc, :],
                        start=(dc == 0),
                        stop=(dc == 3),
                    )
                _copy(
                    nc, CFG["v"],
                    st_["v"][:, stt, :, 0:64],
                    vp_ps[:].rearrange("p (h e) -> p h e", h=H),
                )

            def proj_work(b):
                w = [(("load", b), partial(emit_load_x, b))]
                w += [
                    (("xt", b, st), partial(emit_xt_group, b, st))
                    for st in range(4)
                ]
                w += [
                    (("qkt", b, mt), partial(emit_qkt_group, b, mt))
                    for mt in (0, 4, 1, 5, 2, 6, 3, 7)
                ]
                w += [
                    (("v", b, stt), partial(emit_v_group, b, stt))
                    for stt in range(4)
                ]
                return w

            # ---------- attention ----------
            def emit_scores_kt(b, hg, kt, se):
                qkT = state[b]["qkT"]
                h0, h1 = 2 * hg, 2 * hg + 1
                off = 128 * kt
                tag = "att" if CFG["psum"] == "shared" else "stp"
                stp = ps_att.tile([128, 1024], F32, tag=tag, name="stp")
                stp2 = stp[:].rearrange("p (hh q) -> p hh q", hh=2)
                preload = CFG["mask"] == "preload"
                if preload:
                    # causal bias preload for the diagonal block
                    _copy(nc, CFG["preload"], stp2[:, :, off : off + 128], trib[:])
                for hi, h in enumerate((h0, h1)):
                    mtq, poq = h // 2, 64 * (h % 2)
                    mtk, pok = 4 + h // 2, 64 * (h % 2)
                    lhs = qkT[mtk][pok : pok + 64, kt * 128 : (kt + 1) * 128]
                    if preload:
                        nc.tensor.matmul(
                            stp[:, hi * 512 + off : hi * 512 + off + 128],
                            lhs,
                            qkT[mtq][poq : poq + 64, off : off + 128],
                            start=False,
                            stop=True,
                        )
                        if off + 128 < 512:
                            nc.tensor.matmul(
                                stp[:, hi * 512 + off + 128 : hi * 512 + 512],
                                lhs,
                                qkT[mtq][poq : poq + 64, off + 128 : 512],
                                start=True,
                                stop=True,
                            )
                    else:
                        nc.tensor.matmul(
                            stp[:, hi * 512 + off : hi * 512 + 512],
                            lhs,
                            qkT[mtq][poq : poq + 64, off:512],
                            start=True,
                            stop=True,
                        )
                nc.scalar.activation(
                    out=se[:, kt, :, off:],
                    in_=stp2[:, :, off:],
                    func=mybir.ActivationFunctionType.Exp,
                    scale=1.0 / np.sqrt(HD),
                )
                if not preload:
                    getattr(nc, CFG["mask_eng"]).tensor_mul(
                        out=se[:, kt, :, off : off + 128],
                        in0=se[:, kt, :, off : off + 128],
                        in1=tri2[:],
                    )

            def alloc_se():
                return sepool.tile([128, 4, 2, 512], BF16, tag="se", name="se")

            def emit_y(b, hg, se, yp, y_sb, fill=None):
                st_ = state[b]
                dc = hg
                h0, h1 = 2 * hg, 2 * hg + 1
                yp2 = yp[:].rearrange("p (hh x) -> p hh x", hh=2)
                for hi, h in enumerate((h0, h1)):
                    for qt in range(4):
                        base = hi * 512 + qt * 65
                        for kt in range(qt + 1):
                            nc.tensor.matmul(
                                yp[:, base : base + 65],
                                se[:, kt, hi, qt * 128 : (qt + 1) * 128],
                                st_["v"][:, kt, h, 0:65],
                                start=(kt == 0),
                                stop=(kt == qt),
                            )
                    if fill:
                        fill()
                rs = rpool.tile([128, 2, 4], F32, tag="rs", name="rs")
                nc.vector.reciprocal_approx_fast(
                    out=rs[:], in_=yp2[:, :, 64:260:65]
                )
                yb = CFG["ysb_bf16"]
                tp = ps_mm.tile([128, 512], BF16 if yb else F32, tag="mm", name="tpy")
                for qt in range(4):
                    nc.vector.tensor_mul(
                        out=y_sb[qt][
                            :, 128 * hg : 128 * hg + 128
                        ].rearrange("p (hh e) -> p hh e", hh=2),
                        in0=yp2[:, :, qt * 65 : qt * 65 + 64],
                        in1=rs[:, :, qt : qt + 1].broadcast_to([128, 2, 64]),
                    )
                    nc.tensor.transpose(
                        tp[:, qt * 128 : (qt + 1) * 128]
                        if yb
                        else tp[:, qt * 128 : (qt + 1) * 128].bitcast(F32R),
                        y_sb[qt][:, dc * 128 : (dc + 1) * 128],
                        ident_b[:] if yb else ident_r[:],
                    )
                def finish_yt():
                    yt = ytpool.tile(
                        [128, 512], F32R, tag=f"yt{dc}", name=f"yt{dc}"
                    )
                    _copy(nc, CFG["yt"], yt[:], tp[:])
                    st_["ynT"].append(yt)

                if CFG["yt_defer"]:
                    return finish_yt
                finish_yt()
                return None

            def emit_proj_group(b, qt):
                st_ = state[b]
                ynT = st_["ynT"]
                op_ps = ps_mm.tile([128, 512], F32, tag="mm", name="op")
                for dc in range(4):
                    nc.tensor.matmul(
                        op_ps[:],
                        ynT[dc][:, qt * 128 : (qt + 1) * 128],
                        wproj[:, dc, :],
                        start=(dc == 0),
                        stop=(dc == 3),
                    )
                ob_view = out_d.ap()[b].rearrange("(qt p) d -> p qt d", p=128)
                if qt == 0:
                    st_["ob"] = opool.tile([128, 4, 512], F32, tag="ob", name="ob")
                eng = ("scalar", "vector")[qt % 2] if b == B - 1 else CFG["ob"]
                _copy(nc, eng, st_["ob"][:, qt, :], op_ps[:])
                if b == B - 1:
                    # last batch: store each q-tile as soon as it's ready
                    nc.sync.dma_start(
                        out=ob_view[:, qt : qt + 1, :],
                        in_=st_["ob"][:, qt : qt + 1, :],
                    )
                elif qt in (1, 3):
                    # store in halves so the tail overlaps the last proj work
                    nc.sync.dma_start(
                        out=ob_view[:, qt - 1 : qt + 1, :],
                        in_=st_["ob"][:, qt - 1 : qt + 1, :],
                    )

            # ---------- main schedule ----------
            queue = []
            done = set()
            ctr = {"budget": 0, "consumed": 0, "pos": 0}

            def run_item(item):
                done.add(item[0])
                ctr["consumed"] += 1
                item[1]()

            def paced_pop():
                # drain the filler queue evenly across the batch's 24 slots
                ctr["pos"] += 1
                if CFG["paced"]:
                    target = round(ctr["budget"] * ctr["pos"] / 24.0)
                    while ctr["consumed"] < target and queue:
                        run_item(queue.pop(0))
                elif queue:
                    run_item(queue.pop(0))

            def ensure(key):
                # queue order is dependency order; run from the front
                while queue and key not in done:
                    run_item(queue.pop(0))

            def emit_scores(b, hg, fill=None):
                se = alloc_se()
                for kt in range(4):
                    emit_scores_kt(b, hg, kt, se)
                    if fill:
                        fill()
                return se

            # prologue: batch 0's projections run inline, with all four
            # score groups emitted before the first y so the PE has work
            # while the V columns are still in flight on DMA
            se_ready = {}
            w0 = proj_work(0)
            for it in w0[:5]:
                run_item(it)  # load x0 + weight DMAs, xT groups
            for hgp in range(4):
                run_item(w0[5 + 2 * hgp])  # qkT q-tile
                run_item(w0[6 + 2 * hgp])  # qkT k-tile
                se_ready[(0, hgp)] = emit_scores(0, hgp)
            for it in w0[13:]:
                run_item(it)  # V groups
            AHEAD = CFG["ahead"]
            pending_proj = []
            pending_yt = None
            for b in range(B):
                w = proj_work(b + 1) if b + 1 < B else []
                # weave the deferred projection groups between the transpose/
                # qkT chains so the PE has latency-free filler while the
                # psum->sbuf copies drain
                slots = (3, 5, 7, 9)
                for i, it in enumerate(pending_proj):
                    w.insert(min(slots[i], len(w)), it)
                queue += w
                pending_proj = []
                ctr["budget"] = len(queue)
                ctr["consumed"] = 0
                ctr["pos"] = 0
                y_sb = [
                    ypool.tile(
                        [128, 512],
                        BF16 if CFG["ysb_bf16"] else F32R,
                        tag=f"y{qt}",
                        name=f"ysb{qt}",
                    )
                    for qt in range(4)
                ]
                for hg in range(4):
                    yp_tag = "att" if CFG["psum"] == "shared" else "yp"
                    yp_pool = ps_att if CFG["psum"] == "shared" else ps_y
                    if CFG["y_first"]:
                        yp = yp_pool.tile([128, 1024], F32, tag=yp_tag, name="yp")
                        se = se_ready.pop((b, hg))
                        new_yt = emit_y(b, hg, se, yp, y_sb, fill=paced_pop)
                    # top up the scores pipeline to AHEAD groups deep
                    for k in range(1, AHEAD + 1):
                        nb, nhg = divmod(4 * b + hg + k, 4)
                        if nb >= B:
                            break
                        if (nb, nhg) in se_ready:
                            continue
                        ensure(("qkt", nb, 4 + nhg))
                        se_ready[(nb, nhg)] = emit_scores(nb, nhg, fill=paced_pop)
                        break  # at most one new score group per slot
                    if not CFG["y_first"]:
                        yp = yp_pool.tile([128, 1024], F32, tag=yp_tag, name="yp")
                        se = se_ready.pop((b, hg))
                        if pending_yt:
                            # previous head-group's yT copy lands on Act AFTER
                            # this slot's exps so it never delays them
                            pending_yt()
                            pending_yt = None
                        pending_yt = emit_y(b, hg, se, yp, y_sb, fill=paced_pop)
                    else:
                        if pending_yt:
                            pending_yt()
                            pending_yt = None
                        pending_yt = new_yt
                    paced_pop()
                    paced_pop()
                if pending_yt:
                    pending_yt()
                    pending_yt = None
                while queue:
                    run_item(queue.pop(0))
                # this batch's projection is deferred into the next attention
                pending_proj = [
                    (("proj", b, qt), partial(emit_proj_group, b, qt))
                    for qt in range(4)
                ]
            for it in pending_proj:
                run_item(it)

    nc.compile()
    return nc


def kernel(x, qkv_weight, proj_weight):
    if "nc" not in _cache:
        _cache["nc"] = build_nc()
    nc = _cache["nc"]
    in_maps = [
        {
            "x": np.ascontiguousarray(x[m], dtype=np.float32),
            "wqkv": np.ascontiguousarray(qkv_weight[m], dtype=np.float32),
            "wproj": np.ascontiguousarray(proj_weight[m], dtype=np.float32),
        }
        for m in range(M)
    ]
    res = bass_utils.run_bass_kernel_spmd(nc, in_maps, core_ids=list(range(N_CORES)))
    return np.stack([res.results[m]["out"] for m in range(M)]).astype(np.float32)


# revision 38
# speedup vs baseline: 1.0876x; 1.0876x over previous
"""Bass/Tile TRN2 kernel for per-model-batched causal self-attention.

Problem: x[M,B,S,D], qkv_w[M,D,3D], proj_w[M,D,D] -> out[M,B,S,D]
M=8 models sharded across 8 NeuronCores (embarrassingly parallel).

Per-core design (model m), per batch b:
  xT      = PE-transpose(x_b)  (f32r)               [D,S]
  qkT     = wqkv[:, :1024].T-proj (fp32r matmul)    [1024,S] -> bf16 (q^T,k^T)
  V       = x @ wqkv[:, 1024:] (fp32r)              [S,512] -> bf16, +ones col
  st[k,q] = K @ Q^T  (bf16, causal-trimmed)         PSUM f32
  p       = exp(st/8) (ScalarE), diag blocks masked by tri01 mul (Pool)
  y_aug   = p.T @ V_aug (bf16)  -> y[q,d] + softmax sums in col 64 (PSUM)
  y       = y_aug * (1/sums)  row-broadcast multiply (DVE)
  ynT     = PE-transpose(y) (f32r)                  [D,S]
  out     = ynT.T @ wproj (fp32r)

Schedule: score groups run two head-groups ahead of the y phase (se
pipeline) so the ScalarE exp chain never gates the PE; the next batch's
load/transpose/projection groups are interleaved as filler between
score/y units via a keyed work queue (ensure() forces prerequisites
before a score group that consumes them, keeping the in-order PE stream
deadlock-free).  Batch 0 runs a DMA-pipelined prologue: weight chunks
arrive in qkT-group order and all four score groups are emitted before
the first y so the PE stays fed while the V weight columns are still in
flight.  Engine split (GPSIMD cannot touch PSUM on TRN2): DVE does
psum->sbuf copies, reciprocal and the normalize; Act does exp, yT and
out copies; Pool does the SBUF-only causal mask multiplies.  PSUM:
score/y tiles share one 3-deep ring (6 banks) + 2 banks for the
projection/transpose ring.
"""

import sys

if "/opt/trn_rl_repo" not in sys.path:
    sys.path.insert(0, "/opt/trn_rl_repo")

from contextlib import nullcontext
from functools import partial

import numpy as np

import concourse.bass as bass
import concourse.mybir as mybir
import concourse.tile as tile
from concourse import bacc, bass_utils
from concourse.masks import (
    make_identity,
    make_lower_triangular,
    make_upper_triangular,
)

M, B, S, D, H = 8, 4, 512, 512, 8
HD = D // H  # 64
F32 = mybir.dt.float32
F32R = mybir.dt.float32r
BF16 = mybir.dt.bfloat16

N_CORES = 8

# engine assignment for the movable ops (sweepable)
# NOTE: GPSIMD (Pool) cannot access PSUM on TRN2 hardware -- psum-touching
# ops may only go on "vector" (DVE) or "scalar" (Act).
CFG = {
    "norm": "vector",   # y normalization tensor_scalar_mul (reads PSUM)
    "v": "vector",      # V psum->sbuf copy
    "yt": "scalar",     # yT psum->sbuf copy
    "ob": "scalar",     # out psum->sbuf copy
    "preload": "vector",  # causal-bias psum preload
    "mask": "post",  # "preload" (pre-matmul psum bias) or "post" (tri mul)
    "mask_eng": "gpsimd",
    "norm_bcast": True,
    "paced": False,
    "ysb_bf16": False,
    "ahead": 2,
    "yt_defer": False,
    "y_first": False,
    "xt": "vector",    # xT psum->sbuf copy
    "qk": "vector",    # qkT psum->sbuf copy (alt: "alt" = alternate DVE/Act)
    "psum": "shared",  # "shared" (stp+yp one 3-deep ring) or "split"
}

_cache = {}


def _copy(nc, eng, out, in_):
    if eng == "scalar":
        nc.scalar.copy(out=out, in_=in_)
    else:
        getattr(nc, eng).tensor_copy(out=out, in_=in_)


def build_nc(reps=1):
    nc = bacc.Bacc("TRN2", target_bir_lowering=False, debug=False)

    x_d = nc.dram_tensor("x", [B, S, D], F32, kind="ExternalInput")
    wqkv_d = nc.dram_tensor("wqkv", [D, 3 * D], F32, kind="ExternalInput")
    wproj_d = nc.dram_tensor("wproj", [D, D], F32, kind="ExternalInput")
    out_d = nc.dram_tensor("out", [B, S, D], F32, kind="ExternalOutput")

    with tile.TileContext(nc) as tc:
        with (
            tc.tile_pool(name="singles", bufs=1) as singles,
            tc.tile_pool(name="xp", bufs=2) as xpool,
            tc.tile_pool(name="xtp", bufs=2) as xtpool,
            tc.tile_pool(name="qk", bufs=2) as qkpool,
            tc.tile_pool(name="vp", bufs=2) as vpool,
            tc.tile_pool(name="se", bufs=4) as sepool,
            tc.tile_pool(name="yp", bufs=2) as ypool,
            tc.tile_pool(name="ytp", bufs=3) as ytpool,
            tc.tile_pool(name="op", bufs=2) as opool,
            tc.tile_pool(name="rp", bufs=4) as rpool,
            tc.tile_pool(name="ps_mm", bufs=2, space=bass.MemorySpace.PSUM) as ps_mm,
            tc.tile_pool(
                name="ps_att",
                bufs=3 if CFG["psum"] == "shared" else 2,
                space=bass.MemorySpace.PSUM,
            ) as ps_att,
            tc.tile_pool(name="ps_y", bufs=1, space=bass.MemorySpace.PSUM) as ps_y,
        ):
          with tc.For_i(0, reps, 1) if reps > 1 else nullcontext():
            # ---- constants ----
            ident = singles.tile([128, 128], F32)
            make_identity(nc, ident[:])
            ident_r = singles.tile([128, 128], F32R)
            nc.vector.tensor_copy(out=ident_r[:], in_=ident[:])
            ident_b = singles.tile([128, 128], BF16)
            nc.vector.tensor_copy(out=ident_b[:], in_=ident[:])
            # strict-lower (k>q) = -30000 bias tile, x2 heads; accumulated
            # into the diag-block PSUM before the score matmul
            trib = singles.tile([128, 2, 128], F32)
            make_lower_triangular(nc, trib[:, 0, :], val=-30000.0, diag=False)
            nc.gpsimd.tensor_copy(out=trib[:, 1, :], in_=trib[:, 0, :])
            tri2 = singles.tile([128, 2, 128], BF16)  # keep-mask (k<=q), x2
            make_upper_triangular(nc, tri2[:, 0, :], val=1.0, diag=True)
            nc.gpsimd.tensor_copy(out=tri2[:, 1, :], in_=tri2[:, 0, :])

            wqk = [singles.tile([128, 4, 128], F32R, name=f"wqk{mt}") for mt in range(8)]
            wv = singles.tile([128, 4, D], F32R)
            wproj = singles.tile([128, 4, D], F32R)

            state = {}

            # ---------- stage A (loads + projections), as schedulable groups ----
            def emit_load_x(b):
                x_sb = [
                    xpool.tile([128, D], F32R, tag=f"x{stq}", name=f"xsb{stq}")
                    for stq in range(4)
                ]
                for stq in range(4):
                    nc.sync.dma_start(
                        out=x_sb[stq][:],
                        in_=x_d.ap().bitcast(F32R)[b][
                            stq * 128 : (stq + 1) * 128, :
                        ],
                    )
                v_sb = vpool.tile([128, 4, H, 66], BF16, tag="v", name="vsb")
                nc.gpsimd.memset(v_sb[:, :, :, 64:65], 1.0)
                xt = xtpool.tile([128, 4, S], F32R, tag="xt", name="xt")
                state[b] = {"x": x_sb, "xT": xt, "qkT": {}, "v": v_sb, "ynT": []}
                if b == 0:
                    # qk weight columns in per-mt tiles ordered to pipeline
                    # with the qkT projection groups; v columns + wproj last
                    for mt in (0, 4, 1, 5, 2, 6, 3, 7):
                        nc.sync.dma_start(
                            out=wqk[mt][:],
                            in_=wqkv_d.ap()
                            .bitcast(F32R)[:, mt * 128 : (mt + 1) * 128]
                            .rearrange("(c p) o -> p c o", p=128),
                        )
                    nc.sync.dma_start(
                        out=wv[:],
                        in_=wqkv_d.ap().bitcast(F32R)[:, 1024:1536].rearrange(
                            "(c p) o -> p c o", p=128
                        ),
                    )
                    nc.sync.dma_start(
                        out=wproj[:],
                        in_=wproj_d.ap()
                        .bitcast(F32R)
                        .rearrange("(c p) o -> p c o", p=128),
                    )

            def emit_xt_group(b, st):
                # transpose the 4 d-blocks of s-rows [st*128, (st+1)*128)
                st_ = state[b]
                tp = ps_mm.tile([128, 4, 128], F32, tag="mm", name="tpx")
                for dc in range(4):
                    nc.tensor.transpose(
                        tp[:, dc, :].bitcast(F32R),
                        st_["x"][st][:, dc * 128 : (dc + 1) * 128],
                        ident_r[:],
                    )
                _copy(
                    nc, CFG["xt"],
                    st_["xT"][:, :, st * 128 : (st + 1) * 128], tp[:]
                )

            def emit_qkt_group(b, mt):
                st_ = state[b]
                xt = st_["xT"]
                mp = ps_mm.tile([128, 512], F32, tag="mm", name="mp")
                for dc in range(4):
                    nc.tensor.matmul(
                        mp[:],
                        wqk[mt][:, dc, :],
                        xt[:, dc, :],
                        start=(dc == 0),
                        stop=(dc == 3),
                    )
                qk = qkpool.tile([128, 512], BF16, tag=f"qk{mt}", name=f"qk{mt}")
                qeng = CFG["qk"] if CFG["qk"] != "alt" else ("vector", "scalar")[mt % 2]
                _copy(nc, qeng, qk[:], mp[:])
                st_["qkT"][mt] = qk

            def emit_v_group(b, stt):
                st_ = state[b]
                xt = st_["xT"]
                vp_ps = ps_mm.tile([128, 512], F32, tag="mm", name="vp")
                for dc in range(4):
                    nc.tensor.matmul(
                        vp_ps[:],
                        xt[:, dc, stt * 128 : (stt + 1) * 128],
                        wv[:, dc, :],
                        start=(dc == 0),
                        stop=(dc == 3),
                    )
                _copy(
                    nc, CFG["v"],
                    st_["v"][:, stt, :, 0:64],
                    vp_ps[:].rearrange("p (h e) -> p h e", h=H),
                )

            def proj_work(b):
                w = [(("load", b), partial(emit_load_x, b))]
                w += [
                    (("xt", b, st), partial(emit_xt_group, b, st))
                    for st in range(4)
                ]
                w += [
                    (("qkt", b, mt), partial(emit_qkt_group, b, mt))
                    for mt in (0, 4, 1, 5, 2, 6, 3, 7)
                ]
                w += [
                    (("v", b, stt), partial(emit_v_group, b, stt))
                    for stt in range(4)
                ]
                return w

            # ---------- attention ----------
            def emit_scores_kt(b, hg, kt, se):
                qkT = state[b]["qkT"]
                h0, h1 = 2 * hg, 2 * hg + 1
                off = 128 * kt
                tag = "att" if CFG["psum"] == "shared" else "stp"
                stp = ps_att.tile([128, 1024], F32, tag=tag, name="stp")
                stp2 = stp[:].rearrange("p (hh q) -> p hh q", hh=2)
                preload = CFG["mask"] == "preload"
                if preload:
                    # causal bias preload for the diagonal block
                    _copy(nc, CFG["preload"], stp2[:, :, off : off + 128], trib[:])
                for hi, h in enumerate((h0, h1)):
                    mtq, poq = h // 2, 64 * (h % 2)
                    mtk, pok = 4 + h // 2, 64 * (h % 2)
                    lhs = qkT[mtk][pok : pok + 64, kt * 128 : (kt + 1) * 128]
                    if preload:
                        nc.tensor.matmul(
                            stp[:, hi * 512 + off : hi * 512 + off + 128],
                            lhs,
                            qkT[mtq][poq : poq + 64, off : off + 128],
                            start=False,
                            stop=True,
                        )
                        if off + 128 < 512:
                            nc.tensor.matmul(
                                stp[:, hi * 512 + off + 128 : hi * 512 + 512],
                                lhs,
                                qkT[mtq][poq : poq + 64, off + 128 : 512],
                                start=True,
                                stop=True,
                            )
                    else:
                        nc.tensor.matmul(
                            stp[:, hi * 512 + off : hi * 512 + 512],
                            lhs,
                            qkT[mtq][poq : poq + 64, off:512],
                            start=True,
                            stop=True,
                        )
                nc.scalar.activation(
                    out=se[:, kt, :, off:],
                    in_=stp2[:, :, off:],
                    func=mybir.ActivationFunctionType.Exp,
                    scale=1.0 / np.sqrt(HD),
                )
                if not preload:
                    getattr(nc, CFG["mask_eng"]).tensor_mul(
                        out=se[:, kt, :, off : off + 128],
                        in0=se[:, kt, :, off : off + 128],
                        in1=tri2[:],
                    )

            def alloc_se():
                return sepool.tile([128, 4, 2, 512], BF16, tag="se", name="se")

            def emit_y(b, hg, se, yp, y_sb, fill=None):
                st_ = state[b]
                dc = hg
                h0, h1 = 2 * hg, 2 * hg + 1
                yp2 = yp[:].rearrange("p (hh x) -> p hh x", hh=2)
                for hi, h in enumerate((h0, h1)):
                    for qt in range(4):
                        base = hi * 512 + qt * 65
                        for kt in range(qt + 1):
                            nc.tensor.matmul(
                                yp[:, base : base + 65],
                                se[:, kt, hi, qt * 128 : (qt + 1) * 128],
                                st_["v"][:, kt, h, 0:65],
                                start=(kt == 0),
                                stop=(kt == qt),
                            )
                    if fill:
                        fill()
                rs = rpool.tile([128, 2, 4], F32, tag="rs", name="rs")
                nc.vector.reciprocal_approx_fast(
                    out=rs[:], in_=yp2[:, :, 64:260:65]
                )
                yb = CFG["ysb_bf16"]
                tp = ps_mm.tile([128, 512], BF16 if yb else F32, tag="mm", name="tpy")
                for qt in range(4):
                    nc.vector.tensor_mul(
                        out=y_sb[qt][
                            :, 128 * hg : 128 * hg + 128
                        ].rearrange("p (hh e) -> p hh e", hh=2),
                        in0=yp2[:, :, qt * 65 : qt * 65 + 64],
                        in1=rs[:, :, qt : qt + 1].broadcast_to([128, 2, 64]),
                    )
                    nc.tensor.transpose(
                        tp[:, qt * 128 : (qt + 1) * 128]
                        if yb
                        else tp[:, qt * 128 : (qt + 1) * 128].bitcast(F32R),
                        y_sb[qt][:, dc * 128 : (dc + 1) * 128],
                        ident_b[:] if yb else ident_r[:],
                    )
                def finish_yt():
                    yt = ytpool.tile(
                        [128, 512], F32R, tag=f"yt{dc}", name=f"yt{dc}"
                    )
                    _copy(nc, CFG["yt"], yt[:], tp[:])
                    st_["ynT"].append(yt)

                if CFG["yt_defer"]:
                    return finish_yt
                finish_yt()
                return None

            def emit_proj_group(b, qt):
                st_ = state[b]
                ynT = st_["ynT"]
                op_ps = ps_mm.tile([128, 512], F32, tag="mm", name="op")
                for dc in range(4):
                    nc.tensor.matmul(
                        op_ps[:],
                        ynT[dc][:, qt * 128 : (qt + 1) * 128],
                        wproj[:, dc, :],
                        start=(dc == 0),
                        stop=(dc == 3),
                    )
                ob_view = out_d.ap()[b].rearrange("(qt p) d -> p qt d", p=128)
                if qt == 0:
                    st_["ob"] = opool.tile([128, 4, 512], F32, tag="ob", name="ob")
                eng = ("scalar", "vector")[qt % 2] if b == B - 1 else CFG["ob"]
                _copy(nc, eng, st_["ob"][:, qt, :], op_ps[:])
                if b == B - 1:
                    # last batch: store each q-tile as soon as it's ready
                    nc.sync.dma_start(
                        out=ob_view[:, qt : qt + 1, :],
                        in_=st_["ob"][:, qt : qt + 1, :],
                    )
                elif qt in (1, 3):
                    # store in halves so the tail overlaps the last proj work
                    nc.sync.dma_start(
                        out=ob_view[:, qt - 1 : qt + 1, :],
                        in_=st_["ob"][:, qt - 1 : qt + 1, :],
                    )

            # ---------- main schedule ----------
            queue = []
            done = set()
            ctr = {"budget": 0, "consumed": 0, "pos": 0}

            def run_item(item):
                done.add(item[0])
                ctr["consumed"] += 1
                item[1]()

            def paced_pop():
                # drain the filler queue evenly across the batch's 24 slots
                ctr["pos"] += 1
                if CFG["paced"]:
                    target = round(ctr["budget"] * ctr["pos"] / 24.0)
                    while ctr["consumed"] < target and queue:
                        run_item(queue.pop(0))
                elif queue:
                    run_item(queue.pop(0))

            def ensure(key):
                # queue order is dependency order; run from the front
                while queue and key not in done:
                    run_item(queue.pop(0))

            def emit_scores(b, hg, fill=None):
                se = alloc_se()
                for kt in range(4):
                    emit_scores_kt(b, hg, kt, se)
                    if fill:
                        fill()
                return se

            # prologue: batch 0's projections run inline, with all four
            # score groups emitted before the first y so the PE has work
            # while the V columns are still in flight on DMA
            se_ready = {}
            w0 = proj_work(0)
            for it in w0[:5]:
                run_item(it)  # load x0 + weight DMAs, xT groups
            for hgp in range(4):
                run_item(w0[5 + 2 * hgp])  # qkT q-tile
                run_item(w0[6 + 2 * hgp])  # qkT k-tile
                se_ready[(0, hgp)] = emit_scores(0, hgp)
            for it in w0[13:]:
                run_item(it)  # V groups
            AHEAD = CFG["ahead"]
            pending_proj = []
            pending_yt = None
            for b in range(B):
                w = proj_work(b + 1) if b + 1 < B else []
                # weave the deferred projection groups between the transpose/
                # qkT chains so the PE has latency-free filler while the
                # psum->sbuf copies drain
                slots = (3, 5, 7, 9)
                for i, it in enumerate(pending_proj):
                    w.insert(min(slots[i], len(w)), it)
                queue += w
                pending_proj = []
                ctr["budget"] = len(queue)
                ctr["consumed"] = 0
                ctr["pos"] = 0
                y_sb = [
                    ypool.tile(
                        [128, 512],
                        BF16 if CFG["ysb_bf16"] else F32R,
                        tag=f"y{qt}",
                        name=f"ysb{qt}",
                    )
                    for qt in range(4)
                ]
                for hg in range(4):
                    yp_tag = "att" if CFG["psum"] == "shared" else "yp"
                    yp_pool = ps_att if CFG["psum"] == "shared" else ps_y
                    if CFG["y_first"]:
                        yp = yp_pool.tile([128, 1024], F32, tag=yp_tag, name="yp")
                        se = se_ready.pop((b, hg))
                        new_yt = emit_y(b, hg, se, yp, y_sb, fill=paced_pop)
                    # top up the scores pipeline to AHEAD groups deep
                    for k in range(1, AHEAD + 1):
                        nb, nhg = divmod(4 * b + hg + k, 4)
                        if nb >= B:
                            break
                        if (nb, nhg) in se_ready:
                            continue
                        ensure(("qkt", nb, 4 + nhg))
                        se_ready[(nb, nhg)] = emit_scores(nb, nhg, fill=paced_pop)
                        break  # at most one new score group per slot
                    if not CFG["y_first"]:
                        yp = yp_pool.tile([128, 1024], F32, tag=yp_tag, name="yp")
                        se = se_ready.pop((b, hg))
                        if pending_yt:
                            # previous head-group's yT copy lands on Act AFTER
                            # this slot's exps so it never delays them
                            pending_yt()
                            pending_yt = None
                        pending_yt = emit_y(b, hg, se, yp, y_sb, fill=paced_pop)
                    else:
                        if pending_yt:
                            pending_yt()
                            pending_yt = None
                        pending_yt = new_yt
                    paced_pop()
                    paced_pop()
                if pending_yt:
                    pending_yt()
                    pending_yt = None
                while queue:
                    run_item(queue.pop(0))
                # this batch's projection is deferred into the next attention
                pending_proj = [
                    (("proj", b, qt), partial(emit_proj_group, b, qt))
                    for qt in range(4)
                ]
            for it in pending_proj:
                run_item(it)

    nc.compile()
    return nc


def kernel(x, qkv_weight, proj_weight):
    if "nc" not in _cache:
        _cache["nc"] = build_nc()
    nc = _cache["nc"]
    in_maps = [
        {
            "x": np.ascontiguousarray(x[m], dtype=np.float32),
            "wqkv": np.ascontiguousarray(qkv_weight[m], dtype=np.float32),
            "wproj": np.ascontiguousarray(proj_weight[m], dtype=np.float32),
        }
        for m in range(M)
    ]
    res = bass_utils.run_bass_kernel_spmd(nc, in_maps, core_ids=list(range(N_CORES)))
    return np.stack([res.results[m]["out"] for m in range(M)]).astype(np.float32)


# revision 39
# speedup vs baseline: 1.0906x; 1.0028x over previous
"""Bass/Tile TRN2 kernel for per-model-batched causal self-attention.

Problem: x[M,B,S,D], qkv_w[M,D,3D], proj_w[M,D,D] -> out[M,B,S,D]
M=8 models sharded across 8 NeuronCores (embarrassingly parallel).

Per-core design (model m), per batch b:
  xT      = PE-transpose(x_b)  (f32r)               [D,S]
  qkT     = wqkv[:, :1024].T-proj (fp32r matmul)    [1024,S] -> bf16 (q^T,k^T)
  V       = x @ wqkv[:, 1024:] (fp32r)              [S,512] -> bf16, +ones col
  st[k,q] = K @ Q^T  (bf16, causal-trimmed)         PSUM f32
  p       = exp(st/8) (ScalarE), diag blocks masked by tri01 mul (Pool)
  y_aug   = p.T @ V_aug (bf16)  -> y[q,d] + softmax sums in col 64 (PSUM)
  y       = y_aug * (1/sums)  row-broadcast multiply (DVE)
  ynT     = PE-transpose(y) (f32r)                  [D,S]
  out     = ynT.T @ wproj (fp32r)

Schedule: score groups run two head-groups ahead of the y phase (se
pipeline) so the ScalarE exp chain never gates the PE; the next batch's
load/transpose/projection groups are interleaved as filler between
score/y units via a keyed work queue (ensure() forces prerequisites
before a score group that consumes them, keeping the in-order PE stream
deadlock-free).  Batch 0 runs a DMA-pipelined prologue: weight chunks
arrive in qkT-group order and all four score groups are emitted before
the first y so the PE stays fed while the V weight columns are still in
flight.  Engine split (GPSIMD cannot touch PSUM on TRN2): DVE does
psum->sbuf copies, reciprocal and the normalize; Act does exp, yT and
out copies; Pool does the SBUF-only causal mask multiplies.  PSUM:
score/y tiles share one 3-deep ring (6 banks) + 2 banks for the
projection/transpose ring.
"""

import sys

if "/opt/trn_rl_repo" not in sys.path:
    sys.path.insert(0, "/opt/trn_rl_repo")

from contextlib import nullcontext
from functools import partial

import numpy as np

import concourse.bass as bass
import concourse.mybir as mybir
import concourse.tile as tile
from concourse import bacc, bass_utils
from concourse.masks import (
    make_identity,
    make_lower_triangular,
    make_upper_triangular,
)

M, B, S, D, H = 8, 4, 512, 512, 8
HD = D // H  # 64
F32 = mybir.dt.float32
F32R = mybir.dt.float32r
BF16 = mybir.dt.bfloat16

N_CORES = 8

# engine assignment for the movable ops (sweepable)
# NOTE: GPSIMD (Pool) cannot access PSUM on TRN2 hardware -- psum-touching
# ops may only go on "vector" (DVE) or "scalar" (Act).
CFG = {
    "norm": "vector",   # y normalization tensor_scalar_mul (reads PSUM)
    "v": "vector",      # V psum->sbuf copy
    "yt": "scalar",     # yT psum->sbuf copy
    "ob": "scalar",     # out psum->sbuf copy
    "preload": "vector",  # causal-bias psum preload
    "mask": "post",  # "preload" (pre-matmul psum bias) or "post" (tri mul)
    "mask_eng": "gpsimd",
    "norm_bcast": True,
    "paced": False,
    "ysb_bf16": False,
    "ahead": 2,
    "yt_defer": False,
    "y_first": False,
    "xt": "vector",    # xT psum->sbuf copy
    "qk": "vector",    # qkT psum->sbuf copy (alt: "alt" = alternate DVE/Act)
    "psum": "shared",  # "shared" (stp+yp one 3-deep ring) or "split"
}

_cache = {}


def _copy(nc, eng, out, in_):
    if eng == "scalar":
        nc.scalar.copy(out=out, in_=in_)
    else:
        getattr(nc, eng).tensor_copy(out=out, in_=in_)


def build_nc(reps=1):
    nc = bacc.Bacc("TRN2", target_bir_lowering=False, debug=False)

    x_d = nc.dram_tensor("x", [B, S, D], F32, kind="ExternalInput")
    wqkv_d = nc.dram_tensor("wqkv", [D, 3 * D], F32, kind="ExternalInput")
    wproj_d = nc.dram_tensor("wproj", [D, D], F32, kind="ExternalInput")
    out_d = nc.dram_tensor("out", [B, S, D], F32, kind="ExternalOutput")

    with tile.TileContext(nc) as tc:
        with (
            tc.tile_pool(name="singles", bufs=1) as singles,
            tc.tile_pool(name="xp", bufs=2) as xpool,
            tc.tile_pool(name="xtp", bufs=2) as xtpool,
            tc.tile_pool(name="qk", bufs=2) as qkpool,
            tc.tile_pool(name="vp", bufs=2) as vpool,
            tc.tile_pool(name="se", bufs=4) as sepool,
            tc.tile_pool(name="yp", bufs=2) as ypool,
            tc.tile_pool(name="ytp", bufs=3) as ytpool,
            tc.tile_pool(name="op", bufs=2) as opool,
            tc.tile_pool(name="rp", bufs=4) as rpool,
            tc.tile_pool(name="ps_mm", bufs=2, space=bass.MemorySpace.PSUM) as ps_mm,
            tc.tile_pool(
                name="ps_att",
                bufs=3 if CFG["psum"] == "shared" else 2,
                space=bass.MemorySpace.PSUM,
            ) as ps_att,
            tc.tile_pool(name="ps_y", bufs=1, space=bass.MemorySpace.PSUM) as ps_y,
        ):
          with tc.For_i(0, reps, 1) if reps > 1 else nullcontext():
            # ---- constants ----
            ident = singles.tile([128, 128], F32)
            make_identity(nc, ident[:])
            ident_r = singles.tile([128, 128], F32R)
            nc.vector.tensor_copy(out=ident_r[:], in_=ident[:])
            ident_b = singles.tile([128, 128], BF16)
            nc.vector.tensor_copy(out=ident_b[:], in_=ident[:])
            # strict-lower (k>q) = -30000 bias tile, x2 heads; accumulated
            # into the diag-block PSUM before the score matmul
            trib = singles.tile([128, 2, 128], F32)
            make_lower_triangular(nc, trib[:, 0, :], val=-30000.0, diag=False)
            nc.gpsimd.tensor_copy(out=trib[:, 1, :], in_=trib[:, 0, :])
            tri2 = singles.tile([128, 2, 128], BF16)  # keep-mask (k<=q), x2
            make_upper_triangular(nc, tri2[:, 0, :], val=1.0, diag=True)
            nc.gpsimd.tensor_copy(out=tri2[:, 1, :], in_=tri2[:, 0, :])

            wqk = [singles.tile([128, 4, 128], F32R, name=f"wqk{mt}") for mt in range(8)]
            wv = singles.tile([128, 4, D], F32R)
            wproj = singles.tile([128, 4, D], F32R)

            state = {}

            # ---------- stage A (loads + projections), as schedulable groups ----
            def emit_load_x(b):
                x_sb = xpool.tile([128, 4, D], F32R, tag="x", name="xsb")
                if b == 0:
                    # chunked so the first transposes start as data lands
                    for stq in range(4):
                        nc.sync.dma_start(
                            out=x_sb[:, stq, :],
                            in_=x_d.ap().bitcast(F32R)[b][
                                stq * 128 : (stq + 1) * 128, :
                            ],
                        )
                else:
                    # prefetched during the previous batch; one DMA saves
                    # HWDGE and semaphore overhead
                    nc.sync.dma_start(
                        out=x_sb[:],
                        in_=x_d.ap().bitcast(F32R)[b].rearrange(
                            "(st p) d -> p st d", p=128
                        ),
                    )
                v_sb = vpool.tile([128, 4, H, 66], BF16, tag="v", name="vsb")
                nc.gpsimd.memset(v_sb[:, :, :, 64:65], 1.0)
                xt = xtpool.tile([128, 4, S], F32R, tag="xt", name="xt")
                state[b] = {"x": x_sb, "xT": xt, "qkT": {}, "v": v_sb, "ynT": []}
                if b == 0:
                    # qk weight columns in per-mt tiles ordered to pipeline
                    # with the qkT projection groups; v columns + wproj last
                    for mt in (0, 4, 1, 5, 2, 6, 3, 7):
                        nc.sync.dma_start(
                            out=wqk[mt][:],
                            in_=wqkv_d.ap()
                            .bitcast(F32R)[:, mt * 128 : (mt + 1) * 128]
                            .rearrange("(c p) o -> p c o", p=128),
                        )
                    nc.sync.dma_start(
                        out=wv[:],
                        in_=wqkv_d.ap().bitcast(F32R)[:, 1024:1536].rearrange(
                            "(c p) o -> p c o", p=128
                        ),
                    )
                    nc.sync.dma_start(
                        out=wproj[:],
                        in_=wproj_d.ap()
                        .bitcast(F32R)
                        .rearrange("(c p) o -> p c o", p=128),
                    )

            def emit_xt_group(b, st):
                # transpose the 4 d-blocks of s-rows [st*128, (st+1)*128)
                st_ = state[b]
                tp = ps_mm.tile([128, 4, 128], F32, tag="mm", name="tpx")
                for dc in range(4):
                    nc.tensor.transpose(
                        tp[:, dc, :].bitcast(F32R),
                        st_["x"][:, st, dc * 128 : (dc + 1) * 128],
                        ident_r[:],
                    )
                _copy(
                    nc, CFG["xt"],
                    st_["xT"][:, :, st * 128 : (st + 1) * 128], tp[:]
                )

            def emit_qkt_group(b, mt):
                st_ = state[b]
                xt = st_["xT"]
                mp = ps_mm.tile([128, 512], F32, tag="mm", name="mp")
                for dc in range(4):
                    nc.tensor.matmul(
                        mp[:],
                        wqk[mt][:, dc, :],
                        xt[:, dc, :],
                        start=(dc == 0),
                        stop=(dc == 3),
                    )
                qk = qkpool.tile([128, 512], BF16, tag=f"qk{mt}", name=f"qk{mt}")
                qeng = CFG["qk"] if CFG["qk"] != "alt" else ("vector", "scalar")[mt % 2]
                _copy(nc, qeng, qk[:], mp[:])
                st_["qkT"][mt] = qk

            def emit_v_group(b, stt):
                st_ = state[b]
                xt = st_["xT"]
                vp_ps = ps_mm.tile([128, 512], F32, tag="mm", name="vp")
                for dc in range(4):
                    nc.tensor.matmul(
                        vp_ps[:],
                        xt[:, dc, stt * 128 : (stt + 1) * 128],
                        wv[:, dc, :],
                        start=(dc == 0),
                        stop=(dc == 3),
                    )
                _copy(
                    nc, CFG["v"],
                    st_["v"][:, stt, :, 0:64],
                    vp_ps[:].rearrange("p (h e) -> p h e", h=H),
                )

            def proj_work(b):
                w = [(("load", b), partial(emit_load_x, b))]
                w += [
                    (("xt", b, st), partial(emit_xt_group, b, st))
                    for st in range(4)
                ]
                w += [
                    (("qkt", b, mt), partial(emit_qkt_group, b, mt))
                    for mt in (0, 4, 1, 5, 2, 6, 3, 7)
                ]
                w += [
                    (("v", b, stt), partial(emit_v_group, b, stt))
                    for stt in range(4)
                ]
                return w

            # ---------- attention ----------
            def emit_scores_kt(b, hg, kt, se):
                qkT = state[b]["qkT"]
                h0, h1 = 2 * hg, 2 * hg + 1
                off = 128 * kt
                tag = "att" if CFG["psum"] == "shared" else "stp"
                stp = ps_att.tile([128, 1024], F32, tag=tag, name="stp")
                stp2 = stp[:].rearrange("p (hh q) -> p hh q", hh=2)
                preload = CFG["mask"] == "preload"
                if preload:
                    # causal bias preload for the diagonal block
                    _copy(nc, CFG["preload"], stp2[:, :, off : off + 128], trib[:])
                for hi, h in enumerate((h0, h1)):
                    mtq, poq = h // 2, 64 * (h % 2)
                    mtk, pok = 4 + h // 2, 64 * (h % 2)
                    lhs = qkT[mtk][pok : pok + 64, kt * 128 : (kt + 1) * 128]
                    if preload:
                        nc.tensor.matmul(
                            stp[:, hi * 512 + off : hi * 512 + off + 128],
                            lhs,
                            qkT[mtq][poq : poq + 64, off : off + 128],
                            start=False,
                            stop=True,
                        )
                        if off + 128 < 512:
                            nc.tensor.matmul(
                                stp[:, hi * 512 + off + 128 : hi * 512 + 512],
                                lhs,
                                qkT[mtq][poq : poq + 64, off + 128 : 512],
                                start=True,
                                stop=True,
                            )
                    else:
                        nc.tensor.matmul(
                            stp[:, hi * 512 + off : hi * 512 + 512],
                            lhs,
                            qkT[mtq][poq : poq + 64, off:512],
                            start=True,
                            stop=True,
                        )
                nc.scalar.activation(
                    out=se[:, kt, :, off:],
                    in_=stp2[:, :, off:],
                    func=mybir.ActivationFunctionType.Exp,
                    scale=1.0 / np.sqrt(HD),
                )
                if not preload:
                    getattr(nc, CFG["mask_eng"]).tensor_mul(
                        out=se[:, kt, :, off : off + 128],
                        in0=se[:, kt, :, off : off + 128],
                        in1=tri2[:],
                    )

            def alloc_se():
                return sepool.tile([128, 4, 2, 512], BF16, tag="se", name="se")

            def emit_y(b, hg, se, yp, y_sb, fill=None):
                st_ = state[b]
                dc = hg
                h0, h1 = 2 * hg, 2 * hg + 1
                yp2 = yp[:].rearrange("p (hh x) -> p hh x", hh=2)
                for hi, h in enumerate((h0, h1)):
                    for qt in range(4):
                        base = hi * 512 + qt * 65
                        for kt in range(qt + 1):
                            nc.tensor.matmul(
                                yp[:, base : base + 65],
                                se[:, kt, hi, qt * 128 : (qt + 1) * 128],
                                st_["v"][:, kt, h, 0:65],
                                start=(kt == 0),
                                stop=(kt == qt),
                            )
                    if fill:
                        fill()
                rs = rpool.tile([128, 2, 4], F32, tag="rs", name="rs")
                nc.vector.reciprocal_approx_fast(
                    out=rs[:], in_=yp2[:, :, 64:260:65]
                )
                yb = CFG["ysb_bf16"]
                tp = ps_mm.tile([128, 512], BF16 if yb else F32, tag="mm", name="tpy")
                for qt in range(4):
                    nc.vector.tensor_mul(
                        out=y_sb[qt][
                            :, 128 * hg : 128 * hg + 128
                        ].rearrange("p (hh e) -> p hh e", hh=2),
                        in0=yp2[:, :, qt * 65 : qt * 65 + 64],
                        in1=rs[:, :, qt : qt + 1].broadcast_to([128, 2, 64]),
                    )
                    nc.tensor.transpose(
                        tp[:, qt * 128 : (qt + 1) * 128]
                        if yb
                        else tp[:, qt * 128 : (qt + 1) * 128].bitcast(F32R),
                        y_sb[qt][:, dc * 128 : (dc + 1) * 128],
                        ident_b[:] if yb else ident_r[:],
                    )
                def finish_yt():
                    yt = ytpool.tile(
                        [128, 512], F32R, tag=f"yt{dc}", name=f"yt{dc}"
                    )
                    _copy(nc, CFG["yt"], yt[:], tp[:])
                    st_["ynT"].append(yt)

                if CFG["yt_defer"]:
                    return finish_yt
                finish_yt()
                return None

            def emit_proj_group(b, qt):
                st_ = state[b]
                ynT = st_["ynT"]
                op_ps = ps_mm.tile([128, 512], F32, tag="mm", name="op")
                for dc in range(4):
                    nc.tensor.matmul(
                        op_ps[:],
                        ynT[dc][:, qt * 128 : (qt + 1) * 128],
                        wproj[:, dc, :],
                        start=(dc == 0),
                        stop=(dc == 3),
                    )
                ob_view = out_d.ap()[b].rearrange("(qt p) d -> p qt d", p=128)
                if qt == 0:
                    st_["ob"] = opool.tile([128, 4, 512], F32, tag="ob", name="ob")
                eng = ("scalar", "vector")[qt % 2] if b == B - 1 else CFG["ob"]
                _copy(nc, eng, st_["ob"][:, qt, :], op_ps[:])
                if b == B - 1:
                    # last batch: store each q-tile as soon as it's ready
                    nc.sync.dma_start(
                        out=ob_view[:, qt : qt + 1, :],
                        in_=st_["ob"][:, qt : qt + 1, :],
                    )
                elif qt == 3:
                    # earlier batches: latency-irrelevant, one DMA per batch
                    nc.sync.dma_start(out=ob_view[:], in_=st_["ob"][:])

            # ---------- main schedule ----------
            queue = []
            done = set()
            ctr = {"budget": 0, "consumed": 0, "pos": 0}

            def run_item(item):
                done.add(item[0])
                ctr["consumed"] += 1
                item[1]()

            def paced_pop():
                # drain the filler queue evenly across the batch's 24 slots
                ctr["pos"] += 1
                if CFG["paced"]:
                    target = round(ctr["budget"] * ctr["pos"] / 24.0)
                    while ctr["consumed"] < target and queue:
                        run_item(queue.pop(0))
                elif queue:
                    run_item(queue.pop(0))

            def ensure(key):
                # queue order is dependency order; run from the front
                while queue and key not in done:
                    run_item(queue.pop(0))

            def emit_scores(b, hg, fill=None):
                se = alloc_se()
                for kt in range(4):
                    emit_scores_kt(b, hg, kt, se)
                    if fill:
                        fill()
                return se

            # prologue: batch 0's projections run inline, with all four
            # score groups emitted before the first y so the PE has work
            # while the V columns are still in flight on DMA
            se_ready = {}
            w0 = proj_work(0)
            for it in w0[:5]:
                run_item(it)  # load x0 + weight DMAs, xT groups
            for hgp in range(4):
                run_item(w0[5 + 2 * hgp])  # qkT q-tile
                run_item(w0[6 + 2 * hgp])  # qkT k-tile
                se_ready[(0, hgp)] = emit_scores(0, hgp)
            for it in w0[13:]:
                run_item(it)  # V groups
            AHEAD = CFG["ahead"]
            pending_proj = []
            pending_yt = None
            for b in range(B):
                w = proj_work(b + 1) if b + 1 < B else []
                # weave the deferred projection groups between the transpose/
                # qkT chains so the PE has latency-free filler while the
                # psum->sbuf copies drain
                slots = (3, 5, 7, 9)
                for i, it in enumerate(pending_proj):
                    w.insert(min(slots[i], len(w)), it)
                queue += w
                pending_proj = []
                ctr["budget"] = len(queue)
                ctr["consumed"] = 0
                ctr["pos"] = 0
                y_sb = [
                    ypool.tile(
                        [128, 512],
                        BF16 if CFG["ysb_bf16"] else F32R,
                        tag=f"y{qt}",
                        name=f"ysb{qt}",
                    )
                    for qt in range(4)
                ]
                for hg in range(4):
                    yp_tag = "att" if CFG["psum"] == "shared" else "yp"
                    yp_pool = ps_att if CFG["psum"] == "shared" else ps_y
                    if CFG["y_first"]:
                        yp = yp_pool.tile([128, 1024], F32, tag=yp_tag, name="yp")
                        se = se_ready.pop((b, hg))
                        new_yt = emit_y(b, hg, se, yp, y_sb, fill=paced_pop)
                    # top up the scores pipeline to AHEAD groups deep
                    for k in range(1, AHEAD + 1):
                        nb, nhg = divmod(4 * b + hg + k, 4)
                        if nb >= B:
                            break
                        if (nb, nhg) in se_ready:
                            continue
                        ensure(("qkt", nb, 4 + nhg))
                        se_ready[(nb, nhg)] = emit_scores(nb, nhg, fill=paced_pop)
                        break  # at most one new score group per slot
                    if not CFG["y_first"]:
                        yp = yp_pool.tile([128, 1024], F32, tag=yp_tag, name="yp")
                        se = se_ready.pop((b, hg))
                        if pending_yt:
                            # previous head-group's yT copy lands on Act AFTER
                            # this slot's exps so it never delays them
                            pending_yt()
                            pending_yt = None
                        pending_yt = emit_y(b, hg, se, yp, y_sb, fill=paced_pop)
                    else:
                        if pending_yt:
                            pending_yt()
                            pending_yt = None
                        pending_yt = new_yt
                    paced_pop()
                    paced_pop()
                if pending_yt:
                    pending_yt()
                    pending_yt = None
                while queue:
                    run_item(queue.pop(0))
                # this batch's projection is deferred into the next attention
                pending_proj = [
                    (("proj", b, qt), partial(emit_proj_group, b, qt))
                    for qt in range(4)
                ]
            for it in pending_proj:
                run_item(it)

    nc.compile()
    return nc


def kernel(x, qkv_weight, proj_weight):
    if "nc" not in _cache:
        _cache["nc"] = build_nc()
    nc = _cache["nc"]
    in_maps = [
        {
            "x": np.ascontiguousarray(x[m], dtype=np.float32),
            "wqkv": np.ascontiguousarray(qkv_weight[m], dtype=np.float32),
            "wproj": np.ascontiguousarray(proj_weight[m], dtype=np.float32),
        }
        for m in range(M)
    ]
    res = bass_utils.run_bass_kernel_spmd(nc, in_maps, core_ids=list(range(N_CORES)))
    return np.stack([res.results[m]["out"] for m in range(M)]).astype(np.float32)


# revision 42
# speedup vs baseline: 1.1333x; 1.0391x over previous
"""Bass/Tile TRN2 kernel for per-model-batched causal self-attention.

Problem: x[M,B,S,D], qkv_w[M,D,3D], proj_w[M,D,D] -> out[M,B,S,D]
M=8 models sharded across 8 NeuronCores (embarrassingly parallel).

Per-core design (model m), per batch b:
  xT      = PE-transpose(x_b)  (f32r)               [D,S]
  qkT     = wqkv[:, :1024].T-proj (fp32r matmul)    [1024,S] -> bf16 (q^T,k^T)
  V       = x @ wqkv[:, 1024:] (fp32r)              [S,512] -> bf16, +ones col
  st[k,q] = K @ Q^T  (bf16, causal-trimmed)         PSUM f32
  p       = exp(st/8) (ScalarE), diag blocks masked by tri01 mul (Pool)
  y_aug   = p.T @ V_aug (bf16)  -> y[q,d] + softmax sums in col 64 (PSUM)
  y       = y_aug * (1/sums)  row-broadcast multiply (DVE)
  ynT     = PE-transpose(y) (f32r)                  [D,S]
  out     = ynT.T @ wproj (fp32r)

Schedule: score groups run two head-groups ahead of the y phase (se
pipeline) so the ScalarE exp chain never gates the PE; the next batch's
load/transpose/projection groups are interleaved as filler between
score/y units via a keyed work queue (ensure() forces prerequisites
before a score group that consumes them, keeping the in-order PE stream
deadlock-free).  Batch 0 runs a DMA-pipelined prologue: weight chunks
arrive in qkT-group order and all four score groups are emitted before
the first y so the PE stays fed while the V weight columns are still in
flight.  Engine split (GPSIMD cannot touch PSUM on TRN2): DVE does
psum->sbuf copies, reciprocal and the normalize; Act does exp, yT and
out copies; Pool does the SBUF-only causal mask multiplies.  PSUM:
score/y tiles share one 3-deep ring (6 banks) + 2 banks for the
projection/transpose ring.
"""

import sys

if "/opt/trn_rl_repo" not in sys.path:
    sys.path.insert(0, "/opt/trn_rl_repo")

from contextlib import nullcontext
from functools import partial

import numpy as np

import concourse.bass as bass
import concourse.mybir as mybir
import concourse.tile as tile
from concourse import bacc, bass_utils
from concourse.masks import (
    make_identity,
    make_lower_triangular,
    make_upper_triangular,
)

M, B, S, D, H = 8, 4, 512, 512, 8
HD = D // H  # 64
F32 = mybir.dt.float32
F32R = mybir.dt.float32r
BF16 = mybir.dt.bfloat16

N_CORES = 8

# engine assignment for the movable ops (sweepable)
# NOTE: GPSIMD (Pool) cannot access PSUM on TRN2 hardware -- psum-touching
# ops may only go on "vector" (DVE) or "scalar" (Act).
CFG = {
    "norm": "vector",   # y normalization tensor_scalar_mul (reads PSUM)
    "v": "vector",      # V psum->sbuf copy
    "yt": "scalar",     # yT psum->sbuf copy
    "ob": "scalar",     # out psum->sbuf copy
    "preload": "vector",  # causal-bias psum preload
    "mask": "post",  # "preload" (pre-matmul psum bias) or "post" (tri mul)
    "mask_eng": "gpsimd",
    "norm_bcast": True,
    "paced": False,
    "ysb_bf16": False,
    "ahead": 2,
    "yt_defer": False,
    "y_first": False,
    "xt": "vector",    # xT psum->sbuf copy
    "qk": "vector",    # qkT psum->sbuf copy (alt: "alt" = alternate DVE/Act)
    "psum": "shared",  # "shared" (stp+yp one 3-deep ring) or "split"
}

_cache = {}


def _copy(nc, eng, out, in_):
    if eng == "scalar":
        nc.scalar.copy(out=out, in_=in_)
    else:
        getattr(nc, eng).tensor_copy(out=out, in_=in_)


def build_nc(reps=1):
    nc = bacc.Bacc("TRN2", target_bir_lowering=False, debug=False)

    x_d = nc.dram_tensor("x", [B, S, D], F32, kind="ExternalInput")
    wqkv_d = nc.dram_tensor("wqkv", [D, 3 * D], F32, kind="ExternalInput")
    wproj_d = nc.dram_tensor("wproj", [D, D], F32, kind="ExternalInput")
    out_d = nc.dram_tensor("out", [B, S, D], F32, kind="ExternalOutput")

    with tile.TileContext(nc) as tc:
        with (
            tc.tile_pool(name="singles", bufs=1) as singles,
            tc.tile_pool(name="xp", bufs=2) as xpool,
            tc.tile_pool(name="xtp", bufs=2) as xtpool,
            tc.tile_pool(name="qk", bufs=2) as qkpool,
            tc.tile_pool(name="vp", bufs=2) as vpool,
            tc.tile_pool(name="se", bufs=4) as sepool,
            tc.tile_pool(name="yp", bufs=2) as ypool,
            tc.tile_pool(name="ytp", bufs=3) as ytpool,
            tc.tile_pool(name="op", bufs=2) as opool,
            tc.tile_pool(name="rp", bufs=4) as rpool,
            tc.tile_pool(name="ps_mm", bufs=2, space=bass.MemorySpace.PSUM) as ps_mm,
            tc.tile_pool(
                name="ps_att",
                bufs=3 if CFG["psum"] == "shared" else 2,
                space=bass.MemorySpace.PSUM,
            ) as ps_att,
            tc.tile_pool(name="ps_y", bufs=1, space=bass.MemorySpace.PSUM) as ps_y,
        ):
          with tc.For_i(0, reps, 1) if reps > 1 else nullcontext():
            # ---- constants ----
            ident = singles.tile([128, 128], F32)
            make_identity(nc, ident[:])
            ident_r = singles.tile([128, 128], F32R)
            nc.vector.tensor_copy(out=ident_r[:], in_=ident[:])
            ident_b = singles.tile([128, 128], BF16)
            nc.vector.tensor_copy(out=ident_b[:], in_=ident[:])
            # strict-lower (k>q) = -30000 bias tile, x2 heads; accumulated
            # into the diag-block PSUM before the score matmul
            trib = singles.tile([128, 2, 128], F32)
            make_lower_triangular(nc, trib[:, 0, :], val=-30000.0, diag=False)
            nc.gpsimd.tensor_copy(out=trib[:, 1, :], in_=trib[:, 0, :])
            tri2 = singles.tile([128, 2, 128], BF16)  # keep-mask (k<=q), x2
            make_upper_triangular(nc, tri2[:, 0, :], val=1.0, diag=True)
            nc.gpsimd.tensor_copy(out=tri2[:, 1, :], in_=tri2[:, 0, :])

            wqk = [
                singles.tile([128, 4, 128], F32R, name=f"wqk{mt}")
                for mt in range(8)
            ]
            wv = singles.tile([128, 4, D], F32R)
            wproj = singles.tile([128, 4, D], F32R)

            state = {}

            # ---------- stage A (loads + projections), as schedulable groups ----
            def emit_load_x(b):
                x_sb = xpool.tile([128, 4, D], F32R, tag="x", name="xsb")
                if b == 0:
                    # chunked so the first transposes start as data lands
                    for stq in range(4):
                        nc.sync.dma_start(
                            out=x_sb[:, stq, :],
                            in_=x_d.ap().bitcast(F32R)[b][
                                stq * 128 : (stq + 1) * 128, :
                            ],
                        )
                else:
                    # prefetched during the previous batch; one DMA saves
                    # HWDGE and semaphore overhead
                    nc.sync.dma_start(
                        out=x_sb[:],
                        in_=x_d.ap().bitcast(F32R)[b].rearrange(
                            "(st p) d -> p st d", p=128
                        ),
                    )
                v_sb = vpool.tile([128, 4, H, 66], BF16, tag="v", name="vsb")
                nc.gpsimd.memset(v_sb[:, :, :, 64:65], 1.0)
                xt = xtpool.tile([128, 4, S], F32R, tag="xt", name="xt")
                state[b] = {"x": x_sb, "xT": xt, "qkT": {}, "v": v_sb, "ynT": []}
                if b == 0:
                    # qk weight columns in per-mt tiles ordered to pipeline
                    # with the qkT projection groups; v columns + wproj last
                    for mt in (0, 4, 1, 5, 2, 6, 3, 7):
                        nc.sync.dma_start(
                            out=wqk[mt][:],
                            in_=wqkv_d.ap()
                            .bitcast(F32R)[:, mt * 128 : (mt + 1) * 128]
                            .rearrange("(c p) o -> p c o", p=128),
                        )
                    nc.sync.dma_start(
                        out=wv[:],
                        in_=wqkv_d.ap().bitcast(F32R)[:, 1024:1536].rearrange(
                            "(c p) o -> p c o", p=128
                        ),
                    )
                    nc.sync.dma_start(
                        out=wproj[:],
                        in_=wproj_d.ap()
                        .bitcast(F32R)
                        .rearrange("(c p) o -> p c o", p=128),
                    )

            def emit_xt_group(b, st):
                # transpose the 4 d-blocks of s-rows [st*128, (st+1)*128)
                st_ = state[b]
                tp = ps_mm.tile([128, 4, 128], F32, tag="mm", name="tpx")
                for dc in range(4):
                    nc.tensor.transpose(
                        tp[:, dc, :].bitcast(F32R),
                        st_["x"][:, st, dc * 128 : (dc + 1) * 128],
                        ident_r[:],
                    )
                _copy(
                    nc, CFG["xt"],
                    st_["xT"][:, :, st * 128 : (st + 1) * 128], tp[:]
                )

            def emit_qkt_group(b, mt):
                st_ = state[b]
                xt = st_["xT"]
                mp = ps_mm.tile([128, 512], F32, tag="mm", name="mp")
                for dc in range(4):
                    nc.tensor.matmul(
                        mp[:],
                        wqk[mt][:, dc, :],
                        xt[:, dc, :],
                        start=(dc == 0),
                        stop=(dc == 3),
                    )
                qk = qkpool.tile([128, 512], BF16, tag=f"qk{mt}", name=f"qk{mt}")
                qeng = CFG["qk"] if CFG["qk"] != "alt" else ("vector", "scalar")[mt % 2]
                _copy(nc, qeng, qk[:], mp[:])
                st_["qkT"][mt] = qk

            def emit_v_group(b, stt):
                st_ = state[b]
                xt = st_["xT"]
                vp_ps = ps_mm.tile([128, 512], F32, tag="mm", name="vp")
                for dc in range(4):
                    nc.tensor.matmul(
                        vp_ps[:],
                        xt[:, dc, stt * 128 : (stt + 1) * 128],
                        wv[:, dc, :],
                        start=(dc == 0),
                        stop=(dc == 3),
                    )
                _copy(
                    nc, CFG["v"],
                    st_["v"][:, stt, :, 0:64],
                    vp_ps[:].rearrange("p (h e) -> p h e", h=H),
                )

            def proj_work(b):
                w = [(("load", b), partial(emit_load_x, b))]
                w += [
                    (("xt", b, st), partial(emit_xt_group, b, st))
                    for st in range(4)
                ]
                w += [
                    (("qkt", b, mt), partial(emit_qkt_group, b, mt))
                    for mt in (0, 4, 1, 5, 2, 6, 3, 7)
                ]
                w += [
                    (("v", b, stt), partial(emit_v_group, b, stt))
                    for stt in range(4)
                ]
                return w

            # ---------- attention ----------
            def emit_scores_kt(b, hg, kt, se):
                qkT = state[b]["qkT"]
                h0, h1 = 2 * hg, 2 * hg + 1
                off = 128 * kt
                tag = "att" if CFG["psum"] == "shared" else "stp"
                stp = ps_att.tile([128, 1024], F32, tag=tag, name="stp")
                stp2 = stp[:].rearrange("p (hh q) -> p hh q", hh=2)
                preload = CFG["mask"] == "preload"
                if preload:
                    # causal bias preload for the diagonal block
                    _copy(nc, CFG["preload"], stp2[:, :, off : off + 128], trib[:])
                for hi, h in enumerate((h0, h1)):
                    mtq, poq = h // 2, 64 * (h % 2)
                    mtk, pok = 4 + h // 2, 64 * (h % 2)
                    lhs = qkT[mtk][pok : pok + 64, kt * 128 : (kt + 1) * 128]
                    if preload:
                        nc.tensor.matmul(
                            stp[:, hi * 512 + off : hi * 512 + off + 128],
                            lhs,
                            qkT[mtq][poq : poq + 64, off : off + 128],
                            start=False,
                            stop=True,
                        )
                        if off + 128 < 512:
                            nc.tensor.matmul(
                                stp[:, hi * 512 + off + 128 : hi * 512 + 512],
                                lhs,
                                qkT[mtq][poq : poq + 64, off + 128 : 512],
                                start=True,
                                stop=True,
                            )
                    else:
                        nc.tensor.matmul(
                            stp[:, hi * 512 + off : hi * 512 + 512],
                            lhs,
                            qkT[mtq][poq : poq + 64, off:512],
                            start=True,
                            stop=True,
                        )
                nc.scalar.activation(
                    out=se[:, kt, :, off:],
                    in_=stp2[:, :, off:],
                    func=mybir.ActivationFunctionType.Exp,
                    scale=1.0 / np.sqrt(HD),
                )
                if not preload:
                    getattr(nc, CFG["mask_eng"]).tensor_mul(
                        out=se[:, kt, :, off : off + 128],
                        in0=se[:, kt, :, off : off + 128],
                        in1=tri2[:],
                    )

            def alloc_se():
                return sepool.tile([128, 4, 2, 512], BF16, tag="se", name="se")

            def emit_y(b, hg, se, yp, y_sb, fill=None):
                st_ = state[b]
                dc = hg
                h0, h1 = 2 * hg, 2 * hg + 1
                yp2 = yp[:].rearrange("p (hh x) -> p hh x", hh=2)
                for hi, h in enumerate((h0, h1)):
                    for qt in range(4):
                        base = hi * 512 + qt * 65
                        for kt in range(qt + 1):
                            nc.tensor.matmul(
                                yp[:, base : base + 65],
                                se[:, kt, hi, qt * 128 : (qt + 1) * 128],
                                st_["v"][:, kt, h, 0:65],
                                start=(kt == 0),
                                stop=(kt == qt),
                            )
                    if fill:
                        fill()
                rs = rpool.tile([128, 2, 4], F32, tag="rs", name="rs")
                nc.vector.reciprocal_approx_fast(
                    out=rs[:], in_=yp2[:, :, 64:260:65]
                )
                yb = CFG["ysb_bf16"]
                tp = ps_mm.tile([128, 512], BF16 if yb else F32, tag="mm", name="tpy")
                # one strided multiply normalizes all four q-tiles
                nc.vector.tensor_mul(
                    out=y_sb[:, :, 128 * hg : 128 * hg + 128].rearrange(
                        "p qt (hh e) -> p hh qt e", hh=2
                    ),
                    in0=yp2[:, :, 0:260]
                    .rearrange("p hh (qt e) -> p hh qt e", e=65)[:, :, :, 0:64],
                    in1=rs[:]
                    .rearrange("p hh qt -> p hh qt ()")
                    .broadcast_to([128, 2, 4, 64]),
                )
                for qt in range(4):
                    nc.tensor.transpose(
                        tp[:, qt * 128 : (qt + 1) * 128]
                        if yb
                        else tp[:, qt * 128 : (qt + 1) * 128].bitcast(F32R),
                        y_sb[:, qt, dc * 128 : (dc + 1) * 128],
                        ident_b[:] if yb else ident_r[:],
                    )
                def finish_yt():
                    yt = ytpool.tile(
                        [128, 512], F32R, tag=f"yt{dc}", name=f"yt{dc}"
                    )
                    _copy(nc, CFG["yt"], yt[:], tp[:])
                    st_["ynT"].append(yt)

                if CFG["yt_defer"]:
                    return finish_yt
                finish_yt()
                return None

            def emit_proj_group(b, qt):
                st_ = state[b]
                ynT = st_["ynT"]
                op_ps = ps_mm.tile([128, 512], F32, tag="mm", name="op")
                for dc in range(4):
                    nc.tensor.matmul(
                        op_ps[:],
                        ynT[dc][:, qt * 128 : (qt + 1) * 128],
                        wproj[:, dc, :],
                        start=(dc == 0),
                        stop=(dc == 3),
                    )
                ob_view = out_d.ap()[b].rearrange("(qt p) d -> p qt d", p=128)
                if qt == 0:
                    st_["ob"] = opool.tile([128, 4, 512], F32, tag="ob", name="ob")
                eng = ("scalar", "vector")[qt % 2] if b == B - 1 else CFG["ob"]
                _copy(nc, eng, st_["ob"][:, qt, :], op_ps[:])
                if b == B - 1:
                    # last batch: store each q-tile as soon as it's ready
                    nc.sync.dma_start(
                        out=ob_view[:, qt : qt + 1, :],
                        in_=st_["ob"][:, qt : qt + 1, :],
                    )
                elif qt == 3:
                    # earlier batches: latency-irrelevant, one DMA per batch
                    nc.sync.dma_start(out=ob_view[:], in_=st_["ob"][:])

            # ---------- main schedule ----------
            queue = []
            done = set()
            ctr = {"budget": 0, "consumed": 0, "pos": 0}

            def run_item(item):
                done.add(item[0])
                ctr["consumed"] += 1
                item[1]()

            def paced_pop():
                # drain the filler queue evenly across the batch's 24 slots
                ctr["pos"] += 1
                if CFG["paced"]:
                    target = round(ctr["budget"] * ctr["pos"] / 24.0)
                    while ctr["consumed"] < target and queue:
                        run_item(queue.pop(0))
                elif queue:
                    run_item(queue.pop(0))

            def ensure(key):
                # queue order is dependency order; run from the front
                while queue and key not in done:
                    run_item(queue.pop(0))

            def emit_scores(b, hg, fill=None):
                se = alloc_se()
                for kt in range(4):
                    emit_scores_kt(b, hg, kt, se)
                    if fill:
                        fill()
                return se

            # prologue: batch 0's projections run inline, with all four
            # score groups emitted before the first y so the PE has work
            # while the V columns are still in flight on DMA
            se_ready = {}
            w0 = proj_work(0)
            for it in w0[:5]:
                run_item(it)  # load x0 + weight DMAs, xT groups
            for hgp in range(4):
                run_item(w0[5 + 2 * hgp])  # qkT q-tile
                run_item(w0[6 + 2 * hgp])  # qkT k-tile
                se_ready[(0, hgp)] = emit_scores(0, hgp)
            for it in w0[13:]:
                run_item(it)  # V groups
            AHEAD = CFG["ahead"]
            pending_proj = []
            pending_yt = None
            for b in range(B):
                w = proj_work(b + 1) if b + 1 < B else []
                # weave the deferred projection groups between the transpose/
                # qkT chains so the PE has latency-free filler while the
                # psum->sbuf copies drain
                slots = (3, 5, 7, 9)
                for i, it in enumerate(pending_proj):
                    w.insert(min(slots[i], len(w)), it)
                queue += w
                pending_proj = []
                ctr["budget"] = len(queue)
                ctr["consumed"] = 0
                ctr["pos"] = 0
                y_sb = ypool.tile(
                    [128, 4, 512],
                    BF16 if CFG["ysb_bf16"] else F32R,
                    tag="y",
                    name="ysb",
                )
                for hg in range(4):
                    yp_tag = "att" if CFG["psum"] == "shared" else "yp"
                    yp_pool = ps_att if CFG["psum"] == "shared" else ps_y
                    if CFG["y_first"]:
                        yp = yp_pool.tile([128, 1024], F32, tag=yp_tag, name="yp")
                        se = se_ready.pop((b, hg))
                        new_yt = emit_y(b, hg, se, yp, y_sb, fill=paced_pop)
                    # top up the scores pipeline to AHEAD groups deep
                    for k in range(1, AHEAD + 1):
                        nb, nhg = divmod(4 * b + hg + k, 4)
                        if nb >= B:
                            break
                        if (nb, nhg) in se_ready:
                            continue
                        ensure(("qkt", nb, 4 + nhg))
                        se_ready[(nb, nhg)] = emit_scores(nb, nhg, fill=paced_pop)
                        break  # at most one new score group per slot
                    if not CFG["y_first"]:
                        yp = yp_pool.tile([128, 1024], F32, tag=yp_tag, name="yp")
                        se = se_ready.pop((b, hg))
                        if pending_yt:
                            # previous head-group's yT copy lands on Act AFTER
                            # this slot's exps so it never delays them
                            pending_yt()
                            pending_yt = None
                        pending_yt = emit_y(b, hg, se, yp, y_sb, fill=paced_pop)
                    else:
                        if pending_yt:
                            pending_yt()
                            pending_yt = None
                        pending_yt = new_yt
                    paced_pop()
                    paced_pop()
                if pending_yt:
                    pending_yt()
                    pending_yt = None
                while queue:
                    run_item(queue.pop(0))
                # this batch's projection is deferred into the next attention
                pending_proj = [
                    (("proj", b, qt), partial(emit_proj_group, b, qt))
                    for qt in range(4)
                ]
            for it in pending_proj:
                run_item(it)

    nc.compile()
    return nc


def kernel(x, qkv_weight, proj_weight):
    if "nc" not in _cache:
        _cache["nc"] = build_nc()
    nc = _cache["nc"]
    in_maps = [
        {
            "x": np.ascontiguousarray(x[m], dtype=np.float32),
            "wqkv": np.ascontiguousarray(qkv_weight[m], dtype=np.float32),
            "wproj": np.ascontiguousarray(proj_weight[m], dtype=np.float32),
        }
        for m in range(M)
    ]
    res = bass_utils.run_bass_kernel_spmd(nc, in_maps, core_ids=list(range(N_CORES)))
    return np.stack([res.results[m]["out"] for m in range(M)]).astype(np.float32)
